# revision 85
# baseline (speedup 1.0000x reference)
"""Trainium2 Bass kernel for batched causal multi-head attention.

Problem: x[B=8,S=1024,D=768], per-head projections W_Q/W_K/W_V [H=12,D,DH=64],
W_O [H,DH,D]; causal softmax attention; output [B,S,D].

Strategy: data-parallel over batch across 8 NeuronCores (no collectives).
Per core (one batch element), computed fully on-chip:
  - Q/K projections in fp8 (e4m3, weights pre-scaled by WSH=64 to dodge
    subnormals; undone in the exp scale) with DoubleRow perf mode: two
    128-deep d-subtiles per matmul = 2x PE throughput. V/scores/z/out stay
    bf16 (fp8 there costs too much accuracy).
  - scores^T [j, i] tiles = kT.T @ qT; causal block-skipping; exp on ScalarE;
    triangular-block mask via a 0/1 mask multiply on DVE.
  - z^T = (v | ones).T @ p^T accumulated over j-tiles in PSUM; the ones column
    yields the softmax denominator as row 64 (no extra matmul).
  - per-(pair, i-block) normalization inline: den rows DMA to partitions 0/32
    (DMA to partitions 64/96 corrupts on HW!), fp32 reciprocal_approx_fast,
    bf16 selector-matmul broadcast (fp32 matmul is 4 cyc/row; fp32r trips the
    BIR verifier), multiply fused into the z bf16-convert copy. GPSIMD
    partition_broadcast measured ~10us on HW - avoided.
  - ONE flat, continuously-staggered score-tile stream across all (pair,
    i-block) boundaries. All non-attention work (V/QK/out projections,
    reciprocals, normalization) drains as small "filler" units into the
    stagger's natural PE stall slots — in-order engines never see a solid
    block of foreign work between two exp ops, so the ScalarE exp stream
    (the phase pacer at ~10us/pair) never starves. q/k prefetched 2 pairs
    ahead; fillers ordered so their PSUM ring slots rotate without stalls.
  - output projection overlaps the last pair's normalization chain via two
    pre-opened accumulation groups (pairs 0-4 emitted early, pr5 + copies
    after), alternating qk/scores PSUM rings; split output DMAs.
Engine balance in the attention phase: ScalarE ~ exp only, DVE ~ qk copies +
zraw/ztmp copies + masks + reciprocals, PE ~ all matmuls; each ~10us/pair.
Host-side prep (free): transpose/pack x and weights into bf16/fp8 SBUF
images with >=512B contiguous DMA elements. `reps`/`loop_reps`/`phases` are
benchmarking aids (static unroll / on-device For_i loop / phase subsetting).
Measured: TimelineSim 119.1us (baseline 152.1); HW loop steady-state ~151-160us
band (baseline 200.7, tunnel drift dominates); rel err 1.05e-2 (gate 2e-2).
"""

import os
from contextlib import ExitStack

import numpy as np

B, S, D, H, DH = 8, 1024, 768, 12, 64
P = 128
DT = 6  # d tiles (D / 128)
ST = 8  # s tiles (S / 128)
PAIRS = 6  # head pairs (H / 2)
NB = 512  # i-block width
WSH = 64.0  # fp8 W_Q/W_K pre-scale (keeps weights out of e4m3 subnormals)
SCALE = (1.0 / 8.0) / (WSH * WSH)  # 1/sqrt(DH), fp8 W pre-scale undone

_CACHE = {}


def _build(qk_bias: bool, v_bias: bool, reps: int = 1, loop_reps: int = 0, phases: str = 'abc', dbg: bool = False):
    import concourse.bass as bass  # noqa: F401
    import concourse.mybir as mybir
    import concourse.tile as tile
    from concourse import bacc

    f32 = mybir.dt.float32
    bf16 = mybir.dt.bfloat16
    fp8 = mybir.dt.float8e4
    Exp = mybir.ActivationFunctionType.Exp
    DoubleRow = mybir.MatmulPerfMode.DoubleRow

    nc = bacc.Bacc("TRN2", target_bir_lowering=False, debug=False)

    xT = nc.dram_tensor("xT", [P, ST, DT, P], bf16, kind="ExternalInput").ap()
    # fp8 copies for the Q/K projections (W pre-scaled by 2^WSH host-side;
    # un-scaled via the exp activation's scale). DoubleRow perf mode pairs two
    # 128-deep d-subtiles per matmul: 2x PE throughput.
    x8 = nc.dram_tensor("x8", [P, DT, S], fp8, kind="ExternalInput").ap()
    wqk = nc.dram_tensor("wqk", [P, PAIRS, 2, DT, P], fp8, kind="ExternalInput").ap()
    wv = nc.dram_tensor("wv", [P, DT, D], bf16, kind="ExternalInput").ap()
    wo = nc.dram_tensor("wo", [P, PAIRS, D], bf16, kind="ExternalInput").ap()
    mask2 = nc.dram_tensor("mask2", [P, 2, P], bf16, kind="ExternalInput").ap()
    if qk_bias:
        bq = nc.dram_tensor("bq", [P, PAIRS], f32, kind="ExternalInput").ap()
        bk = nc.dram_tensor("bk", [P, PAIRS], f32, kind="ExternalInput").ap()
    if v_bias:
        bv = nc.dram_tensor("bv", [1, D], f32, kind="ExternalInput").ap()
    out = nc.dram_tensor("out", [S, D], bf16, kind="ExternalOutput").ap()
    if dbg:
        dbg_qk = nc.dram_tensor("dbg_qk", [P, 2, S], bf16, kind="ExternalOutput").ap()
        dbg_z = nc.dram_tensor("dbg_z", [P, PAIRS, S], bf16, kind="ExternalOutput").ap()
        dbg_rec = nc.dram_tensor("dbg_rec", [33, PAIRS, 2, NB], bf16, kind="ExternalOutput").ap()
        dbg_v = nc.dram_tensor("dbg_v", [P, ST, H, DH + 1], bf16, kind="ExternalOutput").ap()

    def mmr(o, lhsT, rhs, start, stop):
        nc.tensor.matmul(o, lhsT, rhs, start=start, stop=stop)

    with tile.TileContext(nc) as tc:
      with ExitStack() as loop_ctx:
        if loop_reps:
            loop_ctx.enter_context(tc.For_i(0, loop_reps, 1))
        for _rep in range(reps):
          with ExitStack() as ctx:
            consts = ctx.enter_context(tc.tile_pool(name="consts", bufs=1))
            xt_p = ctx.enter_context(tc.tile_pool(name="xt", bufs=1))
            w_p = ctx.enter_context(tc.tile_pool(name="w", bufs=1))
            v_p = ctx.enter_context(tc.tile_pool(name="v", bufs=1))
            z_p = ctx.enter_context(tc.tile_pool(name="z", bufs=1))
            qk_p = ctx.enter_context(tc.tile_pool(name="qk", bufs=4))
            p_p = ctx.enter_context(tc.tile_pool(name="p", bufs=8))
            rec_p = ctx.enter_context(tc.tile_pool(name="rec", bufs=10))
            out_p = ctx.enter_context(tc.tile_pool(name="out", bufs=6))

            # DMA order + chunking: xt/wv gate the first v-proj matmuls, so
            # land them in fine-grained pieces (Tile deps are AP-range aware);
            # wq/wk per pair; wo/mask are needed much later. xT host layout
            # [P, st, dt, c] keeps each chunk's per-partition run contiguous
            # (1536B elements — full DMA bus efficiency).
            xt = xt_p.tile([P, ST, DT, P], bf16)
            wv_t = w_p.tile([P, DT, D], bf16, tag="wv")
            nc.sync.dma_start(out=wv_t[:, 0:1, :], in_=wv[:, 0:1, :])
            nc.sync.dma_start(out=xt[:, 0, 0:3, :], in_=xT[:, 0, 0:3, :])
            nc.sync.dma_start(out=wv_t[:, 1:2, :], in_=wv[:, 1:2, :])
            nc.sync.dma_start(out=xt[:, 0, 3:6, :], in_=xT[:, 0, 3:6, :])
            for dt in range(2, DT):
                nc.sync.dma_start(out=wv_t[:, dt : dt + 1, :], in_=wv[:, dt : dt + 1, :])
            x8_t = xt_p.tile([P, DT, S], fp8, tag="x8")
            wqk_t = w_p.tile([P, PAIRS, 2, DT, P], fp8, tag="wqk")
            for st in range(1, 4):
                nc.sync.dma_start(out=xt[:, st, :, :], in_=xT[:, st, :, :])
            nc.sync.dma_start(out=x8_t[:, 0:2, :], in_=x8[:, 0:2, :])
            for pr2 in range(2):
                nc.sync.dma_start(
                    out=wqk_t[:, pr2 : pr2 + 1, :, :, :], in_=wqk[:, pr2 : pr2 + 1, :, :, :]
                )
            for st in range(4, ST):
                nc.sync.dma_start(out=xt[:, st, :, :], in_=xT[:, st, :, :])
            for tp in range(1, DT // 2):
                nc.sync.dma_start(
                    out=x8_t[:, 2 * tp : 2 * tp + 2, :], in_=x8[:, 2 * tp : 2 * tp + 2, :]
                )
            for pr in range(2, PAIRS):
                nc.sync.dma_start(
                    out=wqk_t[:, pr : pr + 1, :, :, :], in_=wqk[:, pr : pr + 1, :, :, :]
                )
            mask2_t = consts.tile([P, 2, P], bf16)
            nc.sync.dma_start(out=mask2_t[:, :, :], in_=mask2[:, :, :])
            wo_t = w_p.tile([P, PAIRS, D], bf16, tag="wo")
            nc.sync.dma_start(out=wo_t[:, :, :], in_=wo[:, :, :])
            if qk_bias:
                bq_t = consts.tile([P, PAIRS], f32, tag="bq")
                nc.sync.dma_start(out=bq_t[:, :], in_=bq[:, :])
                bk_t = consts.tile([P, PAIRS], f32, tag="bk")
                nc.sync.dma_start(out=bk_t[:, :], in_=bk[:, :])
            if v_bias:
                bv_row = consts.tile([P, D], f32, tag="bvr")
                nc.sync.dma_start(out=bv_row[0:1, :], in_=bv[:, :])
                bv_full = consts.tile([P, D], f32, tag="bvf")
                nc.gpsimd.partition_broadcast(bv_full[:, :], bv_row[0:1, :])

            # v layout: [s-tile, head, 65] — col 64 of each head group is 1.0
            # (ones column makes z-matmul also produce the softmax denominator)
            v_t = v_p.tile([P, ST, H, DH + 1], bf16)
            if 'a' in phases:
                for st in range(ST):
                    nc.vector.memset(v_t[:, st, :, DH], 1.0)
            else:
                nc.vector.memset(v_t[:, :, :, :], 1.0)

            z_t = z_p.tile([P, PAIRS, S], bf16)
            if 'b' not in phases:
                nc.vector.memset(z_t[:, :, :], 0.0)
            # unnormalized-z denominators: partition 32*(2*ib+h2), free slot
            # pr (DMA start partitions must be 32-aligned). Per-pair slices
            # [97, 1, NB] keep the reciprocal's free size at 512 so it runs in
            # ~0.7us inline during phase B. Unused partitions stay 1.0 so the
            # reciprocal is finite (zeroed by the selector matmul anyway).
            # den rows live at partitions 0 (h2=0) and 32 (h2=1) only — DMA
            # writes to SBUF partitions 64/96 come back corrupted on HW —
            # with (pair, ib) along the free dim.
            den4 = z_p.tile([33, PAIRS, 2, NB], f32, tag="den4")
            nc.vector.memset(den4[:, :, :, :], 1.0)
            # bf16 copy of the reciprocal rows (the broadcast matmul runs in
            # bf16: full PE rate; fp32 would be 4 cyc/row and fp32r trips the
            # BIR verifier's rounding check)
            rec_bf = z_p.tile([33, PAIRS, 2, NB], bf16, tag="rec_bf")
            nc.vector.memset(rec_bf[:, :, :, :], 1.0)
            # selector per h2: out rows 0-63 <- rec row 32*h2
            sel4 = consts.tile([33, 2, 64], bf16, tag="sel4")
            nc.vector.memset(sel4[:, :, :], 0.0)
            nc.vector.memset(sel4[0:1, 0, :], 1.0)
            nc.vector.memset(sel4[32:33, 1, :], 1.0)

            # ---------------- Phases A/B/C, interleaved emission -------------
            # One shared general-purpose PSUM pool (tag "qk", [128,512] = 1
            # bank, bufs=2) serves the V projection, the Q/K projections and
            # the output projection, so all phases fit the 8 PSUM banks
            # together and the Tile scheduler can overlap them freely.
            with (
                tc.tile_pool(name="ps_qk", bufs=2, space="PSUM") as ps_qk,
                tc.tile_pool(name="ps_sc", bufs=2, space="PSUM") as ps_sc,
                tc.tile_pool(name="ps_z", bufs=2, space="PSUM") as ps_z,
            ):
                def emit_vproj(st_lo, st_hi):
                    for st in range(st_lo, st_hi):
                        vp1 = ps_qk.tile([P, NB], f32, tag="qk", name="vp1")
                        vp2 = ps_qk.tile([P, NB], f32, tag="qk", name="vp2")
                        for dt in range(DT):
                            mmr(vp1[:, :], xt[:, st, dt, :], wv_t[:, dt, 0:NB],
                                dt == 0, dt == DT - 1)
                        for dt in range(DT):
                            mmr(vp2[:, 0 : D - NB], xt[:, st, dt, :],
                                wv_t[:, dt, NB:D], dt == 0, dt == DT - 1)
                        nc.scalar.copy(
                            v_t[:, st, 0:8, 0:DH],
                            vp1.rearrange("p (h e) -> p h e", e=DH),
                        )
                        nc.scalar.copy(
                            v_t[:, st, 8:12, 0:DH],
                            vp2[:, 0 : D - NB].rearrange("p (h e) -> p h e", e=DH),
                        )
                        if v_bias:
                            nc.vector.tensor_add(
                                v_t[:, st, :, 0:DH],
                                v_t[:, st, :, 0:DH],
                                bv_full.rearrange("p (h e) -> p h e", e=DH),
                            )

                def emit_qkproj(pr):
                    qT_t = qk_p.tile([P, S], bf16, tag="q")
                    kT_t = qk_p.tile([P, S], bf16, tag="k")
                    for iqk, (dst, b_t) in enumerate(((qT_t, "bq"), (kT_t, "bk"))):
                        for ib in range(2):
                            ps = ps_qk.tile([P, NB], f32, tag="qk", name="qkps")
                            for tp in range(DT // 2):
                                nc.tensor.matmul(
                                    ps[:, :],
                                    wqk_t[:, pr, iqk, 2 * tp : 2 * tp + 2, :],
                                    x8_t[:, 2 * tp : 2 * tp + 2, ib * NB : (ib + 1) * NB],
                                    start=tp == 0,
                                    stop=tp == DT // 2 - 1,
                                    perf_mode=DoubleRow,
                                )
                            nc.vector.tensor_copy(
                                dst[:, ib * NB : (ib + 1) * NB], ps[:, :]
                            )
                        if qk_bias:
                            bias_ap = (bq_t if b_t == "bq" else bk_t)[:, pr : pr + 1]
                            nc.vector.tensor_scalar_add(dst[:, :], dst[:, :], bias_ap)
                    return qT_t, kT_t

                # Filler units: small closures (one PSUM group + its copy)
                # drained into the attention loop's natural PE stall slots so
                # projection/normalization work never sits as a solid block
                # between two pairs' exp streams (the in-order engines would
                # stall the exp pacer on it).
                import collections

                fillq = collections.deque()

                def fill(n):
                    for _ in range(min(n, len(fillq))):
                        fillq.popleft()()

                def queue_rec(pr, ib):
                    # per-(pair, ib) reciprocal on that ib's two den rows
                    # (partitions 0 and 32), plus the bf16 copy the broadcast
                    # matmul reads; as fillers so their wait on the den-DMA
                    # semaphore doesn't block the DVE queue.
                    def unit():
                        nc.vector.reciprocal_approx_fast(
                            den4[:, pr, ib, :], den4[:, pr, ib, :]
                        )
                        nc.vector.tensor_copy(
                            rec_bf[:, pr, ib, :], den4[:, pr, ib, :]
                        )
                    fillq.append(unit)

                def queue_norm(pr, ib2, zraws):
                    # normalization units: per h2 one bf16 broadcast-matmul +
                    # fused normalize+bf16 copy + z DMA. The last pair's bc
                    # tiles come from the (idle by then) z ring so phase C's
                    # pre-opened output groups can hold the qk ring.
                    bc_pool, bc_tag = (
                        (ps_z, "z") if pr == PAIRS - 1 else (ps_qk, "qk")
                    )
                    for h2 in range(2):
                        def unit(h2=h2):
                            ztmp = rec_p.tile([64, NB], bf16, tag="ztmp")
                            if 'n' not in phases:
                                bc = bc_pool.tile([P, NB], f32, tag=bc_tag, name="bc")
                                nc.tensor.matmul(
                                    bc[0:64, :],
                                    sel4[:, h2, :],
                                    rec_bf[:, pr, ib2, :],
                                    start=True,
                                    stop=True,
                                )
                                nc.vector.tensor_mul(
                                    ztmp[:, :],
                                    zraws[(ib2, h2)][0:64, :],
                                    bc[0:64, :],
                                )
                            else:
                                nc.vector.tensor_copy(
                                    ztmp[:, :], zraws[(ib2, h2)][0:64, :]
                                )
                            nc.sync.dma_start(
                                z_t[
                                    64 * h2 : 64 * (h2 + 1),
                                    pr,
                                    ib2 * NB : (ib2 + 1) * NB,
                                ],
                                ztmp[:, :],
                            )
                        fillq.append(unit)

                def queue_qkproj(pr):
                    qT_t = qk_p.tile([P, S], bf16, tag="q")
                    kT_t = qk_p.tile([P, S], bf16, tag="k")
                    for iqk, dst in ((0, qT_t), (1, kT_t)):
                        for ib in range(2):
                            def unit(iqk=iqk, dst=dst, ib=ib):
                                ps = ps_qk.tile([P, NB], f32, tag="qk", name="qkps")
                                for tp in range(DT // 2):
                                    nc.tensor.matmul(
                                        ps[:, :],
                                        wqk_t[:, pr, iqk, 2 * tp : 2 * tp + 2, :],
                                        x8_t[:, 2 * tp : 2 * tp + 2,
                                             ib * NB : (ib + 1) * NB],
                                        start=tp == 0,
                                        stop=tp == DT // 2 - 1,
                                        perf_mode=DoubleRow,
                                    )
                                nc.vector.tensor_copy(
                                    dst[:, ib * NB : (ib + 1) * NB], ps[:, :]
                                )
                                if qk_bias and ib == 1:
                                    bias_ap = (bq_t if iqk == 0 else bk_t)[
                                        :, pr : pr + 1
                                    ]
                                    nc.vector.tensor_scalar_add(
                                        dst[:, :], dst[:, :], bias_ap
                                    )
                            fillq.append(unit)
                    return qT_t, kT_t

                def queue_vproj(st_lo, st_hi):
                    for st in range(st_lo, st_hi):
                        def unit(st=st):
                            emit_vproj(st, st + 1)
                        fillq.append(unit)

                # ---- Flat attention stream: one continuously-staggered ----
                # score-tile pipeline across all (pair, i-block) boundaries;
                # block-end work (zraw, den DMA, rec, norm) and projection
                # prefetch drain as fillers in the stagger's stall slots.
                if 'a' in phases:
                    emit_vproj(0, 4)
                qks = {}
                if 'b' in phases:
                    qks[0] = emit_qkproj(0)
                    if 'a' in phases:
                        queue_vproj(4, 8)
                    if PAIRS > 1:
                        qks[1] = queue_qkproj(1)

                tiles = []
                for pr in range(PAIRS if 'b' in phases else 0):
                    for ib in range(2):
                        njt = 4 * (ib + 1)
                        for jt in range(njt):
                            tiles.append((pr, ib, jt, njt))

                zps_cur = {}
                zraws_all = {}

                def emit_z(pr, ib, jt, njt, pt, o):
                    if "zps" not in zps_cur:
                        # allocated lazily so filler bc tiles queued at block
                        # boundaries precede these in the PSUM ring rotation
                        zps_cur["zps"] = [
                            ps_z.tile([P, NB], f32, tag="z", name="zpsA"),
                            ps_z.tile([P, NB], f32, tag="z", name="zpsB"),
                        ]
                    for h2 in range(2):
                        h = 2 * pr + h2
                        mmr(
                            zps_cur["zps"][h2][0 : DH + 1, o:NB],
                            v_t[:, jt, h, :],
                            pt[:, h2, o:NB],
                            jt == 0,
                            jt == njt - 1,
                        )

                def finish_block(pr, ib):
                    # drain the finished block's z accumulators + den rows,
                    # then queue rec + norm units and the next q/k prefetch
                    for h2 in range(2):
                        zraw = rec_p.tile([DH + 1, NB], f32, tag="zraw")
                        nc.vector.tensor_copy(
                            zraw[:, :], zps_cur["zps"][h2][0 : DH + 1, :]
                        )
                        if 'n' not in phases:
                            pp = 32 * h2
                            nc.sync.dma_start(
                                den4[pp : pp + 1, pr, ib, :], zraw[DH : DH + 1, :]
                            )
                        zraws_all[(pr, ib, h2)] = zraw
                    zps_cur.clear()
                    zr = {
                        (ib, h2): zraws_all[(pr, ib, h2)] for h2 in range(2)
                    }
                    # q/k prefetch units first: their DVE copies must clear
                    # well before the next pair's first scores, ahead of the
                    # normalization burst in the queue. Queued at the END of
                    # this pair's FIRST i-block: they then drain across the
                    # second i-block's 8 slots with plenty of slack.
                    if ib == 0 and pr + 2 < PAIRS:
                        qks[pr + 2] = queue_qkproj(pr + 2)
                    if 'n' not in phases:
                        queue_rec(pr, ib)
                    queue_norm(pr, ib, zr)

                prev = None  # (pr, ib, jt, njt, pt, o)
                for pr, ib, jt, njt in tiles:
                    qT_t, kT_t = qks[pr]
                    o = max(0, P * jt - NB * ib)
                    sps = ps_sc.tile([P, 2, NB], f32, tag="sc")
                    for h2 in range(2):
                        mmr(
                            sps[:, h2, o:NB],
                            kT_t[64 * h2 : 64 * (h2 + 1), jt * P : (jt + 1) * P],
                            qT_t[64 * h2 : 64 * (h2 + 1), ib * NB + o : (ib + 1) * NB],
                            True,
                            True,
                        )
                    pt = p_p.tile([P, 2, NB], bf16, tag="p")
                    nc.scalar.activation(
                        pt[:, :, o:NB], sps[:, :, o:NB], Exp, scale=SCALE
                    )
                    if P * jt - NB * ib >= 0:  # diagonal crossing tile
                        nc.vector.tensor_mul(
                            pt[:, :, o : o + P],
                            pt[:, :, o : o + P],
                            mask2_t[:, :, :],
                        )
                    if prev is not None:
                        emit_z(*prev)
                        if prev[2] == prev[3] - 1:  # closed out a block
                            finish_block(prev[0], prev[1])
                    fill(1)
                    prev = (pr, ib, jt, njt, pt, o)
                if prev is not None:
                    emit_z(*prev)
                    finish_block(prev[0], prev[1])

                # Pre-open the first two output s-tiles' accumulation groups
                # (pairs 0..4 only) ahead of the last pair's normalization
                # chain; their PE work hides the den-DMA/reciprocal latency.
                pre = {}
                if 'b' in phases and 'c' in phases and 'n' not in phases:
                    op1 = ps_qk.tile([P, NB], f32, tag="qk", name="op1")
                    op2 = ps_qk.tile([P, NB], f32, tag="qk", name="op2")
                    pre[0] = (op1[:, :], op2[:, 0 : D - NB])
                    # both scores-ring slots are free once the stream ends:
                    # pre-open two more s-tiles there
                    ops = ps_sc.tile([P, 2, NB], f32, tag="sc", name="opsc")
                    pre[1] = (ops[:, 0, :], ops[:, 1, 0 : D - NB])
                    ops3 = ps_sc.tile([P, 2, NB], f32, tag="sc", name="opsc")
                    pre[3] = (ops3[:, 0, :], ops3[:, 1, 0 : D - NB])
                    for st in sorted(pre):
                        o1, o2 = pre[st]
                        for pr in range(PAIRS - 1):
                            lhsT = z_t[:, pr, st * P : (st + 1) * P]
                            mmr(o1, lhsT, wo_t[:, pr, 0:NB], pr == 0, False)
                        for pr in range(PAIRS - 1):
                            lhsT = z_t[:, pr, st * P : (st + 1) * P]
                            mmr(o2, lhsT, wo_t[:, pr, NB:D], pr == 0, False)
                # ---------------- Phase C: output projection -----------------
                # s-tiles 0-3 consume only first-i-block z (ready before the
                # stream ends), so they are emitted BEFORE the flush of the
                # last pair's second-i-block normalization — their PE work
                # hides that chain's den-DMA/reciprocal latency. s-tiles 4-7
                # (which need the flushed z) follow. Rings alternate qk/sc so
                # consecutive s-tiles never wait on each other's PSUM slot.
                def emit_ost(st):
                    if st in pre:
                        o1, o2 = pre[st]
                        lhsT = z_t[:, PAIRS - 1, st * P : (st + 1) * P]
                        mmr(o1, lhsT, wo_t[:, PAIRS - 1, 0:NB], False, True)
                        mmr(o2, lhsT, wo_t[:, PAIRS - 1, NB:D], False, True)
                    else:
                        if st % 2 == 0:
                            op1 = ps_qk.tile([P, NB], f32, tag="qk", name="op1")
                            op2 = ps_qk.tile([P, NB], f32, tag="qk", name="op2")
                            o1, o2 = op1[:, :], op2[:, 0 : D - NB]
                        else:
                            ops = ps_sc.tile([P, 2, NB], f32, tag="sc", name="opsc")
                            o1, o2 = ops[:, 0, :], ops[:, 1, 0 : D - NB]
                        for pr in range(PAIRS):
                            lhsT = z_t[:, pr, st * P : (st + 1) * P]
                            mmr(o1, lhsT, wo_t[:, pr, 0:NB], pr == 0, pr == PAIRS - 1)
                        for pr in range(PAIRS):
                            lhsT = z_t[:, pr, st * P : (st + 1) * P]
                            mmr(o2, lhsT, wo_t[:, pr, NB:D], pr == 0, pr == PAIRS - 1)
                    ot = out_p.tile([P, D], bf16, tag="ot")
                    nc.scalar.copy(ot[:, 0:NB], o1)
                    if st == ST - 1:
                        # last s-tile: ship the first half while the second
                        # half's copy runs (shortens the end-of-kernel chain);
                        # all other s-tiles use one DMA to halve HWDGE load
                        nc.sync.dma_start(
                            out[st * P : (st + 1) * P, 0:NB], ot[:, 0:NB]
                        )
                        nc.vector.tensor_copy(ot[:, NB:D], o2)
                        nc.sync.dma_start(
                            out[st * P : (st + 1) * P, NB:D], ot[:, NB:D]
                        )
                    else:
                        nc.vector.tensor_copy(ot[:, NB:D], o2)
                        nc.sync.dma_start(out[st * P : (st + 1) * P, :], ot[:, :])

                fill(len(fillq))
                if 'c' in phases:
                    for st in range(ST):
                        emit_ost(st)

                if dbg:
                    q5, k5 = qks[PAIRS - 1]
                    nc.sync.dma_start(dbg_qk[:, 0, :], q5[:, :])
                    nc.sync.dma_start(dbg_qk[:, 1, :], k5[:, :])
                    nc.sync.dma_start(dbg_z[:, :, :], z_t[:, :, :])
                    nc.sync.dma_start(dbg_rec[:, :, :, :], rec_bf[:, :, :, :])
                    nc.sync.dma_start(dbg_v[:, :, :, :], v_t[:, :, :, :])

    nc.compile()
    return nc


def _pack_host(inputs):
    import ml_dtypes

    bf = ml_dtypes.bfloat16
    f8 = ml_dtypes.float8_e4m3
    x = np.ascontiguousarray(np.asarray(inputs["normalized_resid_pre"], np.float32))
    WQ = np.asarray(inputs["W_Q"], np.float32)
    WK = np.asarray(inputs["W_K"], np.float32)
    WV = np.asarray(inputs["W_V"], np.float32)
    WO = np.asarray(inputs["W_O"], np.float32)

    def pack_qk(W):
        img = np.empty((P, PAIRS, DT, P), np.float32)
        for pr in range(PAIRS):
            for dt in range(DT):
                img[:, pr, dt, 0:64] = W[2 * pr, dt * P : (dt + 1) * P, :]
                img[:, pr, dt, 64:128] = W[2 * pr + 1, dt * P : (dt + 1) * P, :]
        return np.ascontiguousarray(img)

    # wqk8[p, pr, iqk, dt, c]: fp8 copies of W_Q/W_K scaled by WSH
    wqk_img = np.ascontiguousarray(
        np.stack([pack_qk(WQ * WSH), pack_qk(WK * WSH)], axis=2)
    ).astype(f8)
    # wv_sb[p, dt, n] = WV_flat[dt*128+p, n];  WV_flat[d, h*64+e] = WV[h, d, e]
    wv_flat = WV.transpose(1, 0, 2).reshape(D, D)
    wv_img = np.ascontiguousarray(wv_flat.reshape(DT, P, D).transpose(1, 0, 2)).astype(
        bf
    )
    # wo_sb[p, pr, n]: rows stack the pair's two heads' DH dims
    wo_img = np.ascontiguousarray(WO.reshape(PAIRS, P, D).transpose(1, 0, 2)).astype(bf)
    m = (np.arange(P)[:, None] <= np.arange(P)[None, :]).astype(np.float32)
    mask2_img = np.ascontiguousarray(np.stack([m, m], axis=1)).astype(bf)
    # xT_sb[p, st, dt, c] = x[b][st*128+c, dt*128+p]
    xT_imgs = [
        np.ascontiguousarray(
            x[b].reshape(ST, P, DT, P).transpose(3, 0, 2, 1)
        ).astype(bf)
        for b in range(B)
    ]
    # x8_sb[p, dt, s] = x[b][s, dt*128+p] (fp8, for the Q/K projections)
    x8_imgs = [
        np.ascontiguousarray(x[b].T.reshape(DT, P, S).transpose(1, 0, 2)).astype(f8)
        for b in range(B)
    ]
    return xT_imgs, x8_imgs, wqk_img, wv_img, wo_img, mask2_img


def prepare(inputs):
    """Host-side packing: returns (in_maps, qk_bias, v_bias)."""
    bq_np = np.asarray(inputs["b_Q"], np.float32)
    bk_np = np.asarray(inputs["b_K"], np.float32)
    bv_np = np.asarray(inputs["b_V"], np.float32)
    qk_bias = bool(np.any(bq_np) or np.any(bk_np))
    v_bias = bool(np.any(bv_np))

    xT_imgs, x8_imgs, wqk_img, wv_img, wo_img, mask2_img = _pack_host(inputs)

    common = {
        "wqk": wqk_img,
        "wv": wv_img,
        "wo": wo_img,
        "mask2": mask2_img,
    }
    if qk_bias:
        # biases ride on the WSH-scaled q/k (undone by the exp scale)
        common["bq"] = np.ascontiguousarray(bq_np.reshape(PAIRS, P).T) * WSH
        common["bk"] = np.ascontiguousarray(bk_np.reshape(PAIRS, P).T) * WSH
    if v_bias:
        common["bv"] = np.ascontiguousarray(bv_np.reshape(1, D))

    in_maps = [dict(common, xT=xT_imgs[b], x8=x8_imgs[b]) for b in range(B)]
    return in_maps, qk_bias, v_bias


def kernel(**inputs):
    global LAST_EXEC_TIME_NS
    from concourse.bass_utils import run_bass_kernel_spmd

    bo_np = np.asarray(inputs["b_O"], np.float32)
    in_maps, qk_bias, v_bias = prepare(inputs)

    reps = int(os.environ.get("KERNEL_REPS", "1"))
    key = (qk_bias, v_bias, reps)
    if key not in _CACHE:
        _CACHE[key] = _build(qk_bias, v_bias, reps)
    nc = _CACHE[key]

    trace = os.environ.get("KERNEL_TRACE", "0") == "1"
    try:
        res = run_bass_kernel_spmd(
            nc, in_maps, core_ids=list(range(B)), trace=trace
        )
    except ModuleNotFoundError:
        # axon NTFF profiling hook unavailable in this container
        res = run_bass_kernel_spmd(nc, in_maps, core_ids=list(range(B)))
    LAST_EXEC_TIME_NS = res.exec_time_ns
    if trace and res.exec_time_ns is not None:
        print(f"HW exec time: {res.exec_time_ns} ns")

    out = np.stack(
        [np.asarray(res.results[b]["out"], np.float32) for b in range(B)], axis=0
    )
    out = out + bo_np[None, None, :]
    return out.astype(np.float32)


LAST_EXEC_TIME_NS = None

